# revision 51
# baseline (speedup 1.0000x reference)
"""PhiHarmonicAttention (B=1, S=2048, D=2048, H=16, Dh=128) on 8 Trainium2 cores.

Sharding: tensor-parallel over heads — 2 heads per core.
  - Wq/Wk/Wv column-sliced (256 cols per core), Wo row-sliced (256 rows).
  - Each core computes q^T/k^T (RoPE'd) + v for its 2 heads, causal
    softmax(QK^T)V in transposed layout, and a partial x-out product with its
    Wo slice. Host sums the 8 partials (TP row-parallel reduction).

All matmuls run in float32r (TF32-like, full PE rate at N>=256,
~1e-4 relative rounding). Scores are computed without max subtraction
(valid: scores ~ N(0,1), |scores| < ~6, exp is safe in fp32).

Pipeline: per 512-wide seq chunk j — projections A(j), then attention
B(h0,j), B(h1,j), then output projection C(j). Causality means B(*,j) only
needs A(0..j), so PE never starves at stage boundaries.
"""
import numpy as np
from contextlib import ExitStack, nullcontext

import concourse.bass as bass
import concourse.tile as tile
from concourse import bacc, mybir
from concourse.bass_utils import run_bass_kernel_spmd

S = 2048
D = 2048
H = 16
DH = 128
NCORES = 8
HPC = H // NCORES          # heads per core = 2
CW = HPC * DH              # weight col-slice per core = 256
NO = D // 128              # contraction chunks = 16
NJ = S // 512              # 512-wide table chunks = 4 (rope tables)
# seq chunks (start, width): smaller first chunks so attention starts before
# the full 12MB of weights+xt(0) lands (startup is DMA-bound)
CHUNKS = [(0, 512), (512, 512), (1024, 512), (1536, 512)]
NB = S // 128              # 128-wide seq blocks = 16
SCALE = float(1.0 / np.sqrt(np.float32(DH)))

ROT_FACTOR = (1.0 + 5.0 ** 0.5) / 2.0 - 1.0
ROPE_BASE = 10000.0

F32 = mybir.dt.float32
F32R = mybir.dt.float32r


def _build_nc(reps=1, stages="ABC"):
    nc = bacc.Bacc("TRN2", target_bir_lowering=False, debug=False, num_devices=NCORES)

    xt_d = nc.dram_tensor("xt", [D, S], F32R, kind="ExternalInput").ap()
    wq_d = nc.dram_tensor("wq", [D, CW], F32R, kind="ExternalInput").ap()
    wk_d = nc.dram_tensor("wk", [D, CW], F32R, kind="ExternalInput").ap()
    wv_d = nc.dram_tensor("wv", [D, CW], F32R, kind="ExternalInput").ap()
    wo_d = nc.dram_tensor("wo", [CW, D], F32R, kind="ExternalInput").ap()
    rcu_d = nc.dram_tensor("ropecu", [DH, 512], F32, kind="ExternalInput").ap()
    rsu_d = nc.dram_tensor("ropesu", [DH, 512], F32, kind="ExternalInput").ap()
    rc512_d = nc.dram_tensor("ropec512", [DH, NJ], F32, kind="ExternalInput").ap()
    rs512_d = nc.dram_tensor("ropes512", [DH, NJ], F32, kind="ExternalInput").ap()
    onc_d = nc.dram_tensor("onescol", [128, 1], F32R, kind="ExternalInput").ap()
    out_d = nc.dram_tensor("out", [S, D], mybir.dt.float16, kind="ExternalOutput").ap()

    with ExitStack() as ctx:
        tc = ctx.enter_context(tile.TileContext(nc))
        consts = ctx.enter_context(tc.tile_pool(name="consts", bufs=1))
        persist = ctx.enter_context(tc.tile_pool(name="persist", bufs=1))
        xw = ctx.enter_context(tc.tile_pool(name="xw", bufs=11))
        ptp = ctx.enter_context(tc.tile_pool(name="ptp", bufs=4))
        work = ctx.enter_context(tc.tile_pool(name="work", bufs=2))
        outp = ctx.enter_context(tc.tile_pool(name="outp", bufs=6))
        ps = ctx.enter_context(tc.tile_pool(name="ps", bufs=8, space="PSUM"))

        # ---- constants (split DMAs so o=0 weight chunks land first) ----
        wq_s = consts.tile([128, NO, CW], F32R, tag="wq")
        wk_s = consts.tile([128, NO, CW], F32R, tag="wk")
        wv_s = consts.tile([128, NO, CW], F32R, tag="wv")
        wo_s = consts.tile([128, HPC, D], F32R, tag="wo")
        rc = consts.tile([DH, S], F32, tag="rc")
        rs = consts.tile([DH, S], F32, tag="rs")
        msk = consts.tile([128, 4, 512], mybir.dt.bfloat16, tag="msk")
        onc = consts.tile([128, 1], F32R, tag="onc")
        rcu = consts.tile([DH, 512], F32, tag="rcu")
        rsu = consts.tile([DH, 512], F32, tag="rsu")
        rc512 = consts.tile([DH, NJ], F32, tag="rc512")
        rs512 = consts.tile([DH, NJ], F32, tag="rs512")
        for o in range(NO):
            nc.scalar.dma_start(wv_s[:, o, :], wv_d[128 * o:128 * (o + 1), :])
            nc.scalar.dma_start(wq_s[:, o, :], wq_d[128 * o:128 * (o + 1), :])
            nc.scalar.dma_start(wk_s[:, o, :], wk_d[128 * o:128 * (o + 1), :])
            if o == 0:
                nc.scalar.dma_start(rcu[:], rcu_d)
                nc.scalar.dma_start(rsu[:], rsu_d)
                nc.scalar.dma_start(rc512[:], rc512_d)
                nc.scalar.dma_start(rs512[:], rs512_d)
        for j in range(NJ):
            sl = slice(512 * j, 512 * (j + 1))
            tm = work.tile([128, 512], F32, tag="t1")
            nc.vector.tensor_scalar_mul(tm[:], rsu[:], rs512[:, j:j + 1])
            nc.vector.scalar_tensor_tensor(
                rc[:, sl], rcu[:], rc512[:, j:j + 1], tm[:],
                mybir.AluOpType.mult, mybir.AluOpType.subtract,
            )
            tm2 = work.tile([128, 512], F32, tag="tsw")
            nc.vector.tensor_scalar_mul(tm2[:], rcu[:], rs512[:, j:j + 1])
            nc.vector.scalar_tensor_tensor(
                rs[:, sl], rsu[:], rc512[:, j:j + 1], tm2[:],
                mybir.AluOpType.mult, mybir.AluOpType.add,
            )
        iot = work.tile([128, 512], F32, tag="t1")
        nc.gpsimd.iota(
            iot[:], pattern=[[1, 512]], base=0, channel_multiplier=-1,
            allow_small_or_imprecise_dtypes=True,
        )
        for r in range(4):
            nc.vector.tensor_scalar(
                msk[:, r, :], iot[:], float(128 * r), None,
                mybir.AluOpType.is_ge,
            )
        nc.scalar.dma_start(onc[:], onc_d)
        nc.scalar.dma_start(wo_s[:], wo_d.rearrange("(h p) n -> p h n", p=128))

        rep_ctx = (
            tc.For_i(
                0, reps, 1,
                hint_engines=tuple(
                    getattr(mybir.EngineType, e)
                    for e in ("PE", "DVE", "Activation", "SP", "Pool")
                ),
            )
            if reps > 1 else nullcontext()
        )
        ctx.enter_context(rep_ctx)

        # ---- persistent per-head tensors ----
        qT = [persist.tile([DH, S], F32R, tag=f"qT{h}", name=f"qT{h}")
              for h in range(HPC)]
        kT = [persist.tile([DH, S], F32R, tag=f"kT{h}", name=f"kT{h}")
              for h in range(HPC)]
        v_sb = persist.tile([128, NB, CW], F32R, tag="v")
        aT = [persist.tile([DH, S], F32R, tag=f"aT{h}", name=f"aT{h}")
              for h in range(HPC)]

        def rope_apply(psum, dst_slice, s0, w):
            cs = rc[:, s0:s0 + w]
            sn = rs[:, s0:s0 + w]
            raw = work.tile([128, 512], F32, tag="raw")
            nc.scalar.copy(raw[:, :w], psum[:])      # frees the PSUM bank fast
            t1 = work.tile([128, 512], F32, tag="t1")
            nc.vector.tensor_mul(t1[:, :w], raw[:, :w], cs)
            tsw = work.tile([128, 512], F32, tag="tsw")
            nc.vector.tensor_copy(tsw[0:64, :w], raw[64:128, :w])
            nc.vector.tensor_copy(tsw[64:128, :w], raw[0:64, :w])
            nc.vector.tensor_mul(tsw[:, :w], tsw[:, :w], sn)
            nc.vector.tensor_add(dst_slice, t1[:, :w], tsw[:, :w])

        def stage_a(ci):
            s0, w = CHUNKS[ci]
            nblk = w // 128
            pq = [ps.tile([128, w], F32, tag="ps", name=f"pq{ci}_{i}")
                  for i in range(HPC)]
            pk = [ps.tile([128, w], F32, tag="ps", name=f"pk{ci}_{i}")
                  for i in range(HPC)]
            pv = [ps.tile([128, CW], F32, tag="ps", name=f"pv{ci}_{i}")
                  for i in range(nblk)]
            for o in range(NO):
                xt_t = xw.tile([128, w], F32R, tag="xt")
                nc.sync.dma_start(
                    xt_t[:], xt_d[128 * o:128 * (o + 1), s0:s0 + w]
                )
                st = dict(start=(o == 0), stop=(o == NO - 1))
                for m4 in range(nblk):
                    nc.tensor.matmul(
                        pv[m4][:], xt_t[:, 128 * m4:128 * (m4 + 1)],
                        wv_s[:, o, :], **st
                    )
                for h in range(HPC):
                    nc.tensor.matmul(
                        pq[h][:], wq_s[:, o, 128 * h:128 * (h + 1)],
                        xt_t[:], **st
                    )
                    nc.tensor.matmul(
                        pk[h][:], wk_s[:, o, 128 * h:128 * (h + 1)],
                        xt_t[:], **st
                    )
            for m4 in range(nblk):
                nc.vector.tensor_copy(v_sb[:, s0 // 128 + m4, :], pv[m4][:])
            rope_apply(pq[0], qT[0][:, s0:s0 + w], s0, w)
            rope_apply(pk[0], kT[0][:, s0:s0 + w], s0, w)
            rope_apply(pq[1], qT[1][:, s0:s0 + w], s0, w)
            rope_apply(pk[1], kT[1][:, s0:s0 + w], s0, w)

        def stage_b(h, ci):
            s0, w = CHUNKS[ci]
            nb = (s0 + w) // 128
            sb0 = s0 // 128
            po = ps.tile([128, w], F32, tag="ps", name=f"po{h}_{ci}")
            pd = ps.tile([1, w], F32, tag="ps", name=f"pd{h}_{ci}")
            for b in range(nb):
                pss = ps.tile([128, w], F32, tag="ps", name=f"pss{h}_{ci}")
                r = b - sb0
                # clip to the causally-valid column range, but keep the moving
                # free dim >= 256 (f32r drops to 1/4 rate below that)
                c0 = 128 * r if 0 < r <= (w - 256) // 128 else 0
                nc.tensor.matmul(
                    pss[:, c0:],
                    kT[h][:, 128 * b:128 * (b + 1)],
                    qT[h][:, s0 + c0:s0 + w],
                    start=True,
                    stop=True,
                )
                pt = ptp.tile([128, 512], F32R, tag="pt")
                c0 = 128 * r if r > 0 else 0
                nc.scalar.activation(
                    pt[:, c0:w], pss[:, c0:], mybir.ActivationFunctionType.Exp,
                    scale=SCALE,
                )
                if r >= 0:
                    nc.vector.tensor_mul(
                        pt[:, c0:w], pt[:, c0:w], msk[:, r, c0:w]
                    )
                stv = dict(start=(b == 0), stop=(b == nb - 1))
                nc.tensor.matmul(pd[:, c0:], onc[:], pt[:, c0:w], **stv)
                nc.tensor.matmul(
                    po[:, c0:], v_sb[:, b, 128 * h:128 * (h + 1)], pt[:, c0:w],
                    **stv
                )
            rec = work.tile([1, 512], F32, tag="rec", bufs=1)
            with nc.allow_low_precision("softmax denom recip"):
                nc.vector.reciprocal(rec[:, :w], pd[:])
            bc = work.tile([128, 512], F32, tag="bc")
            nc.gpsimd.partition_broadcast(bc[:, :w], rec[:, :w])
            nc.vector.tensor_mul(aT[h][:, s0:s0 + w], po[:], bc[:, :w])

        def stage_c(ci):
            s0, w = CHUNKS[ci]
            for m4 in range(w // 128):
                m = s0 // 128 + m4
                for e in range(NJ):
                    pf = ps.tile([128, 512], F32, tag="ps", name=f"pf{j}_{m4}")
                    for h2 in range(HPC):
                        nc.tensor.matmul(
                            pf[:],
                            aT[h2][:, 128 * m:128 * (m + 1)],
                            wo_s[:, h2, 512 * e:512 * (e + 1)],
                            start=(h2 == 0),
                            stop=(h2 == HPC - 1),
                        )
                    ot = outp.tile([128, 512], mybir.dt.float16, tag="ot")
                    if e % 2 == 0:
                        nc.vector.tensor_copy(ot[:], pf[:])
                    else:
                        nc.scalar.copy(ot[:], pf[:])
                    eng = nc.scalar if e % 2 == 0 else nc.sync
                    eng.dma_start(
                        out_d[128 * m:128 * (m + 1), 512 * e:512 * (e + 1)],
                        ot[:],
                    )

        for ci in range(len(CHUNKS)):
            if "A" in stages:
                stage_a(ci)
            if "B" in stages:
                for h in range(HPC):
                    stage_b(h, ci)
            if "C" in stages:
                stage_c(ci)

    nc.compile()
    return nc


def _host_inputs(x, Wq, Wk, Wv, Wo):
    x = np.asarray(x, dtype=np.float32).reshape(S, D)
    xt = np.ascontiguousarray(x.T)

    half = DH // 2
    inv_freq = (
        ROT_FACTOR
        / (ROPE_BASE ** (np.arange(0, half, dtype=np.float32) * 2.0 / DH))
    ).astype(np.float32)
    sgn = np.where(np.arange(DH) < half, -1.0, 1.0).astype(np.float32)[:, None]
    fd = np.concatenate([inv_freq, inv_freq]).astype(np.float32)[:, None]  # [128,1]
    u = np.arange(512, dtype=np.float32)[None, :]
    jj = (512.0 * np.arange(NJ, dtype=np.float32))[None, :]
    ropecu = np.cos(fd * u).astype(np.float32)            # [128, 512]
    ropesu = (sgn * np.sin(fd * u)).astype(np.float32)
    ropec512 = np.cos(fd * jj).astype(np.float32)         # [128, NJ]
    ropes512 = (sgn * np.sin(fd * jj)).astype(np.float32)

    onescol = np.ones((128, 1), dtype=np.float32)

    Wq = np.asarray(Wq, dtype=np.float32)
    Wk = np.asarray(Wk, dtype=np.float32)
    Wv = np.asarray(Wv, dtype=np.float32)
    Wo = np.asarray(Wo, dtype=np.float32)

    in_maps = []
    for c in range(NCORES):
        sl = slice(CW * c, CW * (c + 1))
        in_maps.append(
            {
                "xt": xt,
                "wq": np.ascontiguousarray(Wq[:, sl]),
                "wk": np.ascontiguousarray(Wk[:, sl]),
                "wv": np.ascontiguousarray(Wv[:, sl]),
                "wo": np.ascontiguousarray(Wo[sl, :]),
                "ropecu": ropecu,
                "ropesu": ropesu,
                "ropec512": ropec512,
                "ropes512": ropes512,
                "onescol": onescol,
            }
        )
    return in_maps


_NC_CACHE = None


def kernel(x, Wq, Wk, Wv, Wo):
    global _NC_CACHE
    if _NC_CACHE is None:
        _NC_CACHE = _build_nc()
    in_maps = _host_inputs(x, Wq, Wk, Wv, Wo)
    res = run_bass_kernel_spmd(_NC_CACHE, in_maps, core_ids=list(range(NCORES)))
    out = np.zeros((S, D), dtype=np.float32)
    for r in res.results:
        out += r["out"].astype(np.float32)
    return out.reshape(1, S, D)
